# revision 4
# baseline (speedup 1.0000x reference)
"""Trainium2 Bass kernel for a fake-quantized MLP (qlinear -> gelu -> qlinear).

Reference semantics (B,S,C,H = 32,1024,1024,4096):
    x2d = x.reshape(-1, C)
    h   = round(x2d/sx) @ round(w1/sw1).T * (sx*sw1) + b1 ;  s = max(amax,eps)/127
    g   = gelu(h, exact erf)
    y   = round(g/sh) @ round(w2/sw2).T * (sh*sw2) + b2

Strategy: data-parallel over rows across 8 cores.  Quantized ints fit
exactly in bf16, so matmuls run at full bf16 rate with exact fp32 PSUM
accumulation.  Weight scales need no collective (every core scans the full
weights); the two activation scales (x, gelu output) use tiny
AllReduce(max) collectives.

v2 layout changes vs the first working version:
  - all quantized-tile transposes go through the DMA xbar (SBUF->SBUF for
    x/w1 block transposes, DRAM->SBUF bulk transpose-load for w2) instead
    of PE identity matmuls + ACT copies: keeps PE for real matmuls and
    unclogs the ACT engine that previously gated w1 prep.
  - gelu output h is staged to DRAM in fp16 (not fp32): halves the
    mid-kernel HBM traffic; fp16 rounding of g only perturbs the final
    int8 requant on ~0.1% boundary cases (noise ~1e-3 of output max).
  - w2 amax scan + quant + store of quantized-natural w2q (bf16, DRAM)
    are trickled through phase A; at the phase A->B transition w2qT is
    materialized with 8 bulk DMA transpose-loads that hide under the
    h-scale AllReduce.
  - x amax scan is emitted first so its AllReduce launches as early as
    possible; w1 scan + quant pipeline fills the AllReduce latency.
"""

import sys

import numpy as np

try:
    import concourse.bass as bass
except ImportError:  # pragma: no cover
    sys.path.insert(0, "/opt/trn_rl_repo")
    import concourse.bass as bass

import concourse.mybir as mybir
from contextlib import ExitStack
import concourse.tile as tile
from concourse.bass_utils import run_bass_kernel_spmd

from concourse.bass import _add_dep_helper as _add_dep

F32 = mybir.dt.float32
F16 = mybir.dt.float16
BF16 = mybir.dt.bfloat16
AF = mybir.ActivationFunctionType
ALU = mybir.AluOpType

QP = 127.0
EPS = 1e-6
MAGIC = 12582912.0  # 1.5 * 2**23: fp32 round-to-nearest-even integer trick

# full problem shapes
B, S, C, H = 32, 1024, 1024, 4096
N_CORES = 8


def _split_matmul_waits(nc):
    """This toolchain's walrus codegen allows only ONE sync-wait slot per
    lowered instruction (Matmult waits all land on its LDWEIGHTS since
    --enable-ldw-opt=false; queue DMAs use a single-slot DIRECT2D struct).
    Peel extra waits onto same-engine NoOps inserted just before, except for
    framework-generated drain/barrier instructions which support many."""
    n_split = 0
    for f in nc.m.functions:
        for bb in f.blocks:
            insts = bb.instructions
            out = []
            changed = False
            for inst in insts:
                si = getattr(inst, "sync_info", None)
                if si is not None and si.on_wait and len(si.on_wait) > 1:
                    waits = list(si.on_wait)
                    for k, w in enumerate(waits[:-1]):
                        nop = mybir.InstNoOp(
                            name=f"{inst.name}-wsplit{k}", ins=[], outs=[]
                        )
                        nop.engine = inst.engine
                        nop.sync_info = mybir.SyncInfo(
                            on_wait=[w], on_update=[]
                        )
                        out.append(nop)
                    inst.sync_info = mybir.SyncInfo(
                        on_wait=[waits[-1]], on_update=list(si.on_update or [])
                    )
                    n_split += 1
                    changed = True
                out.append(inst)
            if changed:
                bb.instructions = out
    return n_split


def _dedup_ldweights(nc):
    """Tile legalization emits explicit Ldweights+Matmult pairs, and walrus
    runs with --enable-ldw-opt=false, so every matmul re-streams its
    stationary operand (128 extra PE cycles on a 512-cycle matmul).  Drop an
    Ldweights whose weights AP is identical to the previous one on the PE
    stream (the PE array still holds that stationary); keep its semaphore
    effects on a NoOp."""
    n = 0
    for f in nc.m.functions:
        for bb in f.blocks:
            insts = bb.instructions
            out = []
            last_key = None
            changed = False
            for inst in insts:
                if isinstance(inst, mybir.InstLdweights):
                    key = str(inst.ins[0])
                    if key == last_key:
                        si = getattr(inst, "sync_info", None)
                        if si is not None and (si.on_wait or si.on_update):
                            nop = mybir.InstNoOp(
                                name=inst.name + "-lw", ins=[], outs=[]
                            )
                            nop.engine = inst.engine
                            nop.sync_info = si
                            out.append(nop)
                        n += 1
                        changed = True
                        continue
                    last_key = key
                elif isinstance(inst, mybir.InstMatmult):
                    if inst.is_transpose or getattr(inst, "ldweights", None):
                        last_key = None
                out.append(inst)
            if changed:
                bb.instructions = out
    return n


def build_nc(rows=4096, c=C, h=H, n_cores=N_CORES, gelu="Gelu", split_waits=True):
    """Build the per-core SPMD Bass program.

    rows: rows of x2d handled by each core.
    gelu: "Gelu" (HW ACT table), "Erf" (x*(0.5*erf(x/sqrt2)+0.5)),
          "Identity" (for simulator runs; CoreSim lacks Gelu/Erf).
    """
    assert rows % 512 == 0 and c % 512 == 0 and h % 512 == 0
    nc = bass.Bass()

    x_in = nc.dram_tensor("x", [rows, c], F32, kind="ExternalInput")
    w1_in = nc.dram_tensor("w1", [h, c], F32, kind="ExternalInput")
    b1_in = nc.dram_tensor("b1", [h], F32, kind="ExternalInput")
    w2_in = nc.dram_tensor("w2", [c, h], F32, kind="ExternalInput")
    b2_in = nc.dram_tensor("b2", [c], F32, kind="ExternalInput")
    y_out = nc.dram_tensor("y", [rows, c], F32, kind="ExternalOutput")

    ct = c // 128   # c in 128-blocks
    ht = h // 128   # h in 128-blocks
    groups = [list(range(n_cores))]

    with tile.TileContext(nc) as tc, ExitStack() as top:
        consts = top.enter_context(tc.tile_pool(name="consts", bufs=1))
        scal = top.enter_context(tc.tile_pool(name="scal", bufs=1))
        dram = top.enter_context(tc.tile_pool(name="dram", bufs=1, space="DRAM"))

        ident_f = consts.tile([128, 128], F32)
        from concourse import masks
        masks.make_identity(nc, ident_f[:])

        # b1 as (128, ht): b1_sb[p, jb] = b1[jb*128 + p]
        b1_sb = consts.tile([128, ht], F32)
        nc.sync.dma_start(
            out=b1_sb[:], in_=b1_in.ap().rearrange("(a b) -> b a", b=128)
        )

        magic_b = consts.tile([128, 1], F32)
        nc.vector.memset(magic_b[:], MAGIC)

        # h scratch in DRAM, transposed + fp16: (h, rows)
        h_dram = dram.tile([h, rows], F16)
        # quantized-natural w2 (bf16) staging for the bulk transpose-load
        w2q_dram = dram.tile([c, h], BF16, tag="w2qd")
        # collective bounce buffers (DRAM, non-IO)
        arw_in = dram.tile([2, 1], F32, tag="arwi")
        arx_in = dram.tile([1, 1], F32, tag="arxi")
        arx_out = dram.tile([1, 1], F32, tag="arxo")
        arh_in = dram.tile([1, 1], F32, tag="arhi")
        arh_out = dram.tile([1, 1], F32, tag="arho")

        def _preduce(acc, psR, tag):
            """[128,1] partition-max -> [1,1] via PE transpose + DVE reduce."""
            pt = psR.tile([1, 128], F32, tag=tag)
            nc.tensor.matmul(
                pt[:], lhsT=acc[:], rhs=ident_f[:], start=True, stop=True
            )
            out = scal.tile([1, 1], F32, name=tag + "_r")
            nc.vector.tensor_reduce(
                out=out[:], in_=pt[:], axis=mybir.AxisListType.X, op=ALU.max
            )
            return out

        def _derive(bcast_src_dram, name):
            b = scal.tile([128, 1], F32, name=name + "_b")
            nc.sync.dma_start(out=b[:], in_=bcast_src_dram.to_broadcast((128, 1)))
            s = scal.tile([128, 1], F32, name="s_" + name)
            nc.vector.tensor_scalar(
                out=s[:], in0=b[:], scalar1=EPS, scalar2=float(1.0 / QP),
                op0=ALU.max, op1=ALU.mult,
            )
            inv = scal.tile([128, 1], F32, name="inv_" + name)
            nc.vector.reciprocal(out=inv[:], in_=s[:])
            return s, inv

        # ---------- phase 0: x amax scan FIRST (its AllReduce is the long
        # pole: ~50us mesh latency).  512-row tiles, DMA-bound. ----------
        xmax = scal.tile([128, 1], F32)
        nc.vector.memset(xmax[:], 0.0)
        wmax1 = scal.tile([128, 1], F32)
        nc.vector.memset(wmax1[:], 0.0)
        wmax2 = scal.tile([128, 1], F32)
        nc.vector.memset(wmax2[:], 0.0)

        with tc.tile_pool(name="scan", bufs=4) as scanp, tc.tile_pool(
            name="scant", bufs=4
        ) as scant, tc.tile_pool(name="psR", bufs=2, space="PSUM") as psR0:
            for mb in range(rows // 512):
                t = scanp.tile([128, c * 4], F32, tag="sc")
                nc.sync.dma_start(
                    out=t[:].rearrange("b (a c) -> b a c", a=4),
                    in_=x_in[mb * 512 : (mb + 1) * 512, :].rearrange(
                        "(a b) c -> b a c", b=128
                    ),
                )
                r = scant.tile([128, 1], F32, tag="scr")
                nc.vector.tensor_reduce(
                    out=r[:], in_=t[:], axis=mybir.AxisListType.X, op=ALU.max,
                    apply_absolute_value=True,
                )
                nc.vector.tensor_tensor(
                    out=xmax[:], in0=xmax[:], in1=r[:], op=ALU.max
                )
            xmax_r = _preduce(xmax, psR0, "xm")
            nc.gpsimd.dma_start(out=arx_in[:], in_=xmax_r[:])
            nc.gpsimd.collective_compute(
                "AllReduce", ALU.max, replica_groups=groups,
                ins=[arx_in.opt()], outs=[arx_out.opt()],
            )

            # w1 amax scan (same pool).  Full weights per core -> local max
            # is global, no collective.
            for rb in range(h // 512):
                t = scanp.tile([128, c * 4], F32, tag="sc")
                nc.sync.dma_start(
                    out=t[:].rearrange("b (a c) -> b a c", a=4),
                    in_=w1_in[rb * 512 : (rb + 1) * 512, :].rearrange(
                        "(a b) c -> b a c", b=128
                    ),
                )
                r = scant.tile([128, 1], F32, tag="scr")
                nc.vector.tensor_reduce(
                    out=r[:], in_=t[:], axis=mybir.AxisListType.X, op=ALU.max,
                    apply_absolute_value=True,
                )
                nc.vector.tensor_tensor(
                    out=wmax1[:], in0=wmax1[:], in1=r[:], op=ALU.max
                )
            w1max_r = _preduce(wmax1, psR0, "w1m")
            nc.gpsimd.dma_start(out=arw_in[0:1, :], in_=w1max_r[:])

        sw1, inv_sw1 = _derive(arw_in[0:1, :], "w1")

        # ---------- w1 quant + DMA-xbar transpose -> w1qT[cb] (128, h) ----
        # Strip-pipelined so phase A's jb=0 matmuls can start as soon as the
        # first strips land.
        w1_stack = ExitStack()
        w1qT_pool = w1_stack.enter_context(
            tc.tile_pool(name="w1qT", bufs=ct, side="right")
        )
        w1qT = [
            w1qT_pool.tile([128, h], BF16, tag="w1qT", name=f"w1qT{i}")
            for i in range(ct)
        ]
        with tc.tile_pool(name="w1f", bufs=6) as w1f, tc.tile_pool(
            name="w1q", bufs=6
        ) as w1qp:
            for rb in range(ht):
                wt = w1f.tile([128, c], F32, tag="w1f")
                nc.sync.dma_start(
                    out=wt[:], in_=w1_in[rb * 128 : (rb + 1) * 128, :]
                )
                nc.scalar.activation(
                    out=wt[:], in_=wt[:], func=AF.Identity, bias=magic_b[:],
                    scale=inv_sw1[:],
                )
                wq = w1qp.tile([128, c], BF16, tag="w1q")
                nc.vector.tensor_scalar_add(out=wq[:], in0=wt[:], scalar1=-MAGIC)
                for cb in range(ct):
                    nc.sync.dma_start(
                        out=w1qT[cb][:, rb * 128 : (rb + 1) * 128],
                        in_=wq[:, cb * 128 : (cb + 1) * 128],
                        transpose=True,
                    )

        sx, inv_sx = _derive(arx_out, "x")
        sxw1 = scal.tile([128, 1], F32)
        nc.vector.tensor_tensor(out=sxw1[:], in0=sx[:], in1=sw1[:], op=ALU.mult)

        hmax = scal.tile([128, 1], F32)
        nc.vector.memset(hmax[:], 0.0)

        # ---------- phase A: h.T = gelu(w1q @ xq.T * (sx*sw1) + b1) -------
        CH = min(1024, rows)
        n_ms = CH // 512
        n_chunks_a = rows // CH
        # w2 trickle plan: amax-scan strip-halves in the first half of
        # phase A, quant+store strip-halves in the second half.  Strips are
        # processed as (row-block, col-half) pieces of [128, h/2] to keep
        # the SBUF working set small.
        assert n_chunks_a >= 2
        hw2 = h // 2
        n_w2_pieces = ct * 2
        w2_scan_per_chunk = (n_w2_pieces + (n_chunks_a // 2) - 1) // (n_chunks_a // 2)
        w2_q_per_chunk = w2_scan_per_chunk
        sw2_state = {}

        with tc.tile_pool(name="w2s", bufs=3) as w2s, tc.tile_pool(
            name="w2sr", bufs=3
        ) as w2sr, tc.tile_pool(name="w2qf", bufs=2) as w2qf, tc.tile_pool(
            name="w2qb", bufs=2
        ) as w2qb, tc.tile_pool(name="xa", bufs=3) as xa, tc.tile_pool(
            name="xq", bufs=3
        ) as xqp, tc.tile_pool(name="xqt", bufs=2) as xqtp, tc.tile_pool(
            name="gs", bufs=8
        ) as gs, tc.tile_pool(name="gr", bufs=8) as gr, tc.tile_pool(
            name="psH", bufs=3 * n_ms, space="PSUM"
        ) as psH, tc.tile_pool(name="psRw", bufs=1, space="PSUM") as psRw:
            for mc in range(n_chunks_a):
                # --- w2 trickle ---
                if mc < n_chunks_a // 2:
                    for k in range(w2_scan_per_chunk):
                        pc = mc * w2_scan_per_chunk + k
                        if pc < n_w2_pieces:
                            ob, hhalf = pc // 2, pc % 2
                            wt = w2s.tile([128, hw2], F32, tag="w2s")
                            nc.sync.dma_start(
                                out=wt[:],
                                in_=w2_in[ob * 128 : (ob + 1) * 128,
                                          hhalf * hw2 : (hhalf + 1) * hw2],
                            )
                            wr = w2sr.tile([128, 1], F32, tag="w2sr")
                            nc.vector.tensor_reduce(
                                out=wr[:], in_=wt[:], axis=mybir.AxisListType.X,
                                op=ALU.max, apply_absolute_value=True,
                            )
                            nc.vector.tensor_tensor(
                                out=wmax2[:], in0=wmax2[:], in1=wr[:], op=ALU.max
                            )
                else:
                    if "inv_sw2" not in sw2_state:
                        w2max_r = _preduce(wmax2, psRw, "w2m")
                        nc.gpsimd.dma_start(out=arw_in[1:2, :], in_=w2max_r[:])
                        sw2_state["sw2"], sw2_state["inv_sw2"] = _derive(
                            arw_in[1:2, :], "w2"
                        )
                    inv_sw2 = sw2_state["inv_sw2"]
                    for k in range(w2_q_per_chunk):
                        pc = (mc - n_chunks_a // 2) * w2_q_per_chunk + k
                        if pc < n_w2_pieces:
                            ob, hhalf = pc // 2, pc % 2
                            wt = w2qf.tile([128, hw2], F32, tag="w2qf")
                            nc.sync.dma_start(
                                out=wt[:],
                                in_=w2_in[ob * 128 : (ob + 1) * 128,
                                          hhalf * hw2 : (hhalf + 1) * hw2],
                            )
                            nc.scalar.activation(
                                out=wt[:], in_=wt[:], func=AF.Identity,
                                bias=magic_b[:], scale=inv_sw2[:],
                            )
                            wq = w2qb.tile([128, hw2], BF16, tag="w2qb")
                            nc.vector.tensor_scalar_add(
                                out=wq[:], in0=wt[:], scalar1=-MAGIC
                            )
                            nc.sync.dma_start(
                                out=w2q_dram[ob * 128 : (ob + 1) * 128,
                                             hhalf * hw2 : (hhalf + 1) * hw2],
                                in_=wq[:],
                            )

                # --- x quant + xbar transpose for this chunk ---
                xqT = xqtp.tile([128, ct, CH], BF16, tag="xqT", name=f"xqT{mc}")
                for t8 in range(CH // 128):
                    m0 = mc * CH + t8 * 128
                    xt = xa.tile([128, c], F32, tag="xa")
                    nc.sync.dma_start(out=xt[:], in_=x_in[m0 : m0 + 128, :])
                    nc.scalar.activation(
                        out=xt[:], in_=xt[:], func=AF.Identity, bias=magic_b[:],
                        scale=inv_sx[:],
                    )
                    xq = xqp.tile([128, c], BF16, tag="xq")
                    nc.vector.tensor_scalar_add(out=xq[:], in0=xt[:], scalar1=-MAGIC)
                    for cb in range(ct):
                        nc.sync.dma_start(
                            out=xqT[:, cb, t8 * 128 : (t8 + 1) * 128],
                            in_=xq[:, cb * 128 : (cb + 1) * 128],
                            transpose=True,
                        )

                # --- matmul over j blocks; gelu; amax; store h.T fp16 ---
                for jb in range(ht):
                    phs = [
                        psH.tile([128, 512], F32, tag="psH", name=f"psH{mc}_{jb}_{i}")
                        for i in range(n_ms)
                    ]
                    prev = None
                    for cb in range(ct):
                        for ms in range(n_ms):
                            mmi = nc.tensor.matmul(
                                phs[ms][:],
                                lhsT=w1qT[cb][:, jb * 128 : (jb + 1) * 128],
                                rhs=xqT[:, cb, ms * 512 : (ms + 1) * 512],
                                start=(cb == 0),
                                stop=(cb == ct - 1),
                            )
                            if prev is not None:
                                _add_dep(mmi.ins, prev.ins, sync=False,
                                         reason="ldw-order")
                            prev = mmi
                    for ms in range(n_ms):
                        ph = phs[ms]
                        g = gs.tile([128, 512], F16, tag="gs")
                        if gelu == "Erf":
                            hh = gs.tile([128, 512], F32, tag="gh")
                            nc.scalar.activation(
                                out=hh[:], in_=ph[:], func=AF.Identity,
                                bias=b1_sb[:, jb : jb + 1], scale=sxw1[:],
                            )
                            e = gs.tile([128, 512], F32, tag="ge")
                            nc.scalar.activation(
                                out=e[:], in_=hh[:], func=AF.Erf, bias=0.0,
                                scale=float(1.0 / np.sqrt(2.0)),
                            )
                            nc.vector.tensor_scalar(
                                out=e[:], in0=e[:], scalar1=0.5, scalar2=0.5,
                                op0=ALU.mult, op1=ALU.add,
                            )
                            nc.vector.tensor_tensor(
                                out=g[:], in0=e[:], in1=hh[:], op=ALU.mult
                            )
                        else:
                            nc.scalar.activation(
                                out=g[:], in_=ph[:], func=getattr(AF, gelu),
                                bias=b1_sb[:, jb : jb + 1], scale=sxw1[:],
                            )
                        r = gr.tile([128, 1], F32, tag="gr")
                        nc.vector.tensor_reduce(
                            out=r[:], in_=g[:], axis=mybir.AxisListType.X,
                            op=ALU.max, apply_absolute_value=True,
                        )
                        nc.vector.tensor_tensor(
                            out=hmax[:], in0=hmax[:], in1=r[:], op=ALU.max
                        )
                        m0 = mc * CH + ms * 512
                        nc.sync.dma_start(
                            out=h_dram[jb * 128 : (jb + 1) * 128, m0 : m0 + 512],
                            in_=g[:],
                        )

        w1_stack.close()
        sw2 = sw2_state["sw2"]

        # ---------- h scale: AllReduce ----------
        with tc.tile_pool(name="psRh", bufs=2, space="PSUM") as psRh:
            hmax_r = _preduce(hmax, psRh, "hm")
        nc.gpsimd.dma_start(out=arh_in[:], in_=hmax_r[:])
        nc.gpsimd.collective_compute(
            "AllReduce", ALU.max, replica_groups=groups,
            ins=[arh_in.opt()], outs=[arh_out.opt()],
        )

        # ---------- w2qT materialization: bulk DMA transpose-loads --------
        # (rides under the AllReduce above)
        w2_stack = ExitStack()
        w2qT_pool = w2_stack.enter_context(tc.tile_pool(name="w2qT", bufs=1))
        w2qTs = w2qT_pool.tile([128, ht, c], BF16, tag="w2qTs")
        for jg in range(ht // 4):
            nc.sync.dma_start_transpose(
                out=w2qTs[:, jg * 4 : (jg + 1) * 4, :],
                in_=w2q_dram[:, jg * 512 : (jg + 1) * 512],
            )

        sh, inv_sh = _derive(arh_out, "h")
        shw2 = scal.tile([128, 1], F32)
        nc.vector.tensor_tensor(out=shw2[:], in0=sh[:], in1=sw2[:], op=ALU.mult)

        # ---------- phase B: y = hq.T.T @ w2q.T * (sh*sw2) + b2 ----------
        n_chunk = rows // 512
        with tc.tile_pool(name="b2p", bufs=1) as b2p, tc.tile_pool(
            name="hb", bufs=8
        ) as hb, tc.tile_pool(name="hf", bufs=4) as hf, tc.tile_pool(
            name="hqt", bufs=2
        ) as hqtp, tc.tile_pool(name="ys", bufs=4) as ys, tc.tile_pool(
            name="psY", bufs=3 * (c // 512), space="PSUM"
        ) as psY:
            b2_b = b2p.tile([128, c], F32)
            nc.sync.dma_start(
                out=b2_b[:],
                in_=b2_in.ap().rearrange("(o a) -> o a", o=1).to_broadcast((128, c)),
            )

            for mc in range(n_chunk):
                hqT = hqtp.tile([128, ht * 512], BF16, tag="hqT")
                for jb in range(ht):
                    th = hb.tile([128, 512], F16, tag="hb")
                    nc.sync.dma_start(
                        out=th[:],
                        in_=h_dram[jb * 128 : (jb + 1) * 128,
                                   mc * 512 : (mc + 1) * 512],
                    )
                    tf = hf.tile([128, 512], F32, tag="hf")
                    nc.scalar.activation(
                        out=tf[:], in_=th[:], func=AF.Identity, bias=magic_b[:],
                        scale=inv_sh[:],
                    )
                    nc.vector.tensor_scalar_add(
                        out=hqT[:, jb * 512 : (jb + 1) * 512], in0=tf[:],
                        scalar1=-MAGIC,
                    )
                n_ob = c // 512
                for ms in range(4):
                    pys = [
                        psY.tile([128, 512], F32, tag="psY", name=f"psY{mc}_{ms}_{i}")
                        for i in range(n_ob)
                    ]
                    prev = None
                    for jb in range(ht):
                        for ob in range(n_ob):
                            mmi = nc.tensor.matmul(
                                pys[ob][:],
                                lhsT=hqT[:, jb * 512 + ms * 128 :
                                         jb * 512 + (ms + 1) * 128],
                                rhs=w2qTs[:, jb, ob * 512 : (ob + 1) * 512],
                                start=(jb == 0),
                                stop=(jb == ht - 1),
                            )
                            if prev is not None:
                                _add_dep(mmi.ins, prev.ins, sync=False,
                                         reason="ldw-order")
                            prev = mmi
                    for ob in range(n_ob):
                        yt = ys.tile([128, 512], F32, tag="ys")
                        nc.vector.scalar_tensor_tensor(
                            out=yt[:], in0=pys[ob][:], scalar=shw2[:],
                            in1=b2_b[:, ob * 512 : (ob + 1) * 512],
                            op0=ALU.mult, op1=ALU.add,
                        )
                        m0 = mc * 512 + ms * 128
                        nc.sync.dma_start(
                            out=y_out[m0 : m0 + 128, ob * 512 : (ob + 1) * 512],
                            in_=yt[:],
                        )

        w2_stack.close()

    if split_waits:
        _split_matmul_waits(nc)
        _dedup_ldweights(nc)
    return nc


_CACHED = {}


def _get_nc(rows, c, h, n_cores, gelu):
    key = (rows, c, h, n_cores, gelu)
    if key not in _CACHED:
        _CACHED[key] = build_nc(rows=rows, c=c, h=h, n_cores=n_cores, gelu=gelu)
    return _CACHED[key]


def run(inputs, trace=False, gelu="Gelu", n_cores=N_CORES):
    x = np.asarray(inputs["x"], np.float32)
    w1 = np.ascontiguousarray(np.asarray(inputs["w1"], np.float32))
    b1 = np.ascontiguousarray(np.asarray(inputs["b1"], np.float32))
    w2 = np.ascontiguousarray(np.asarray(inputs["w2"], np.float32))
    b2 = np.ascontiguousarray(np.asarray(inputs["b2"], np.float32))
    b_, s_, c_ = x.shape
    h_ = w1.shape[0]
    x2d = np.ascontiguousarray(x.reshape(-1, c_))
    rows = x2d.shape[0] // n_cores
    nc = _get_nc(rows, c_, h_, n_cores, gelu)
    in_maps = [
        {
            "x": np.ascontiguousarray(x2d[i * rows : (i + 1) * rows]),
            "w1": w1,
            "b1": b1,
            "w2": w2,
            "b2": b2,
        }
        for i in range(n_cores)
    ]
    res = run_bass_kernel_spmd(nc, in_maps, list(range(n_cores)), trace=trace)
    y2d = np.concatenate([r["y"] for r in res.results], axis=0)
    return y2d.reshape(b_, s_, c_).astype(np.float32), res


def kernel(x, w1, b1, w2, b2):
    y, _ = run({"x": x, "w1": w1, "b1": b1, "w2": w2, "b2": b2})
    return y


# revision 12
# speedup vs baseline: 1.4526x; 1.4526x over previous
"""Trainium2 Bass kernel for a fake-quantized MLP (qlinear -> gelu -> qlinear).

Reference semantics (B,S,C,H = 32,1024,1024,4096):
    x2d = x.reshape(-1, C)
    h   = round(x2d/sx) @ round(w1/sw1).T * (sx*sw1) + b1 ;  s = max(amax,eps)/127
    g   = gelu(h, exact erf)
    y   = round(g/sh) @ round(w2/sw2).T * (sh*sw2) + b2

Strategy: data-parallel over rows across 8 cores.  Quantized ints fit
exactly in bf16, so matmuls run at full bf16 rate with exact fp32 PSUM
accumulation.  Weight scales need no collective (every core scans the full
weights); the two activation scales (x, gelu output) use tiny
AllReduce(max) collectives.

v2 layout changes vs the first working version:
  - all quantized-tile transposes go through the DMA xbar (SBUF->SBUF for
    x/w1 block transposes, DRAM->SBUF bulk transpose-load for w2) instead
    of PE identity matmuls + ACT copies: keeps PE for real matmuls and
    unclogs the ACT engine that previously gated w1 prep.
  - gelu output h is staged to DRAM in fp16 (not fp32): halves the
    mid-kernel HBM traffic; fp16 rounding of g only perturbs the final
    int8 requant on ~0.1% boundary cases (noise ~1e-3 of output max).
  - w2 amax scan + quant + store of quantized-natural w2q (bf16, DRAM)
    are trickled through phase A; at the phase A->B transition w2qT is
    materialized with 8 bulk DMA transpose-loads that hide under the
    h-scale AllReduce.
  - x amax scan is emitted first so its AllReduce launches as early as
    possible; w1 scan + quant pipeline fills the AllReduce latency.
"""

import sys

import numpy as np

try:
    import concourse.bass as bass
except ImportError:  # pragma: no cover
    sys.path.insert(0, "/opt/trn_rl_repo")
    import concourse.bass as bass

import concourse.mybir as mybir
from contextlib import ExitStack
import concourse.tile as tile
from concourse.bass_utils import run_bass_kernel_spmd

from concourse.bass import _add_dep_helper as _add_dep

F32 = mybir.dt.float32
F16 = mybir.dt.float16
BF16 = mybir.dt.bfloat16
AF = mybir.ActivationFunctionType
ALU = mybir.AluOpType

QP = 127.0
EPS = 1e-6
MAGIC = 12582912.0  # 1.5 * 2**23: fp32 round-to-nearest-even integer trick

# full problem shapes
B, S, C, H = 32, 1024, 1024, 4096
N_CORES = 8


def _split_matmul_waits(nc):
    """This toolchain's walrus codegen allows only ONE sync-wait slot per
    lowered instruction (Matmult waits all land on its LDWEIGHTS since
    --enable-ldw-opt=false; queue DMAs use a single-slot DIRECT2D struct).
    Peel extra waits onto same-engine NoOps inserted just before, except for
    framework-generated drain/barrier instructions which support many."""
    n_split = 0
    for f in nc.m.functions:
        for bb in f.blocks:
            insts = bb.instructions
            out = []
            changed = False
            for inst in insts:
                si = getattr(inst, "sync_info", None)
                if si is not None and si.on_wait and len(si.on_wait) > 1:
                    waits = list(si.on_wait)
                    for k, w in enumerate(waits[:-1]):
                        nop = mybir.InstNoOp(
                            name=f"{inst.name}-wsplit{k}", ins=[], outs=[]
                        )
                        nop.engine = inst.engine
                        nop.sync_info = mybir.SyncInfo(
                            on_wait=[w], on_update=[]
                        )
                        out.append(nop)
                    inst.sync_info = mybir.SyncInfo(
                        on_wait=[waits[-1]], on_update=list(si.on_update or [])
                    )
                    n_split += 1
                    changed = True
                out.append(inst)
            if changed:
                bb.instructions = out
    return n_split


def _dedup_ldweights(nc):
    """Tile legalization emits explicit Ldweights+Matmult pairs, and walrus
    runs with --enable-ldw-opt=false, so every matmul re-streams its
    stationary operand (128 extra PE cycles on a 512-cycle matmul).  Drop an
    Ldweights whose weights AP is identical to the previous one on the PE
    stream (the PE array still holds that stationary); keep its semaphore
    effects on a NoOp."""
    n = 0
    for f in nc.m.functions:
        for bb in f.blocks:
            insts = bb.instructions
            out = []
            last_key = None
            changed = False
            for inst in insts:
                if isinstance(inst, mybir.InstLdweights):
                    key = str(inst.ins[0])
                    if key == last_key:
                        si = getattr(inst, "sync_info", None)
                        if si is not None and (si.on_wait or si.on_update):
                            nop = mybir.InstNoOp(
                                name=inst.name + "-lw", ins=[], outs=[]
                            )
                            nop.engine = inst.engine
                            nop.sync_info = si
                            out.append(nop)
                        n += 1
                        changed = True
                        continue
                    last_key = key
                elif isinstance(inst, mybir.InstMatmult):
                    if inst.is_transpose or getattr(inst, "ldweights", None):
                        last_key = None
                out.append(inst)
            if changed:
                bb.instructions = out
    return n


def build_nc(rows=4096, c=C, h=H, n_cores=N_CORES, gelu="Gelu", split_waits=True):
    """Build the per-core SPMD Bass program.

    rows: rows of x2d handled by each core.
    gelu: "Gelu" (HW ACT table), "Erf" (x*(0.5*erf(x/sqrt2)+0.5)),
          "Identity" (for simulator runs; CoreSim lacks Gelu/Erf).
    """
    assert rows % 512 == 0 and c % 512 == 0 and h % 512 == 0
    nc = bass.Bass()

    x_in = nc.dram_tensor("x", [rows, c], F32, kind="ExternalInput")
    w1_in = nc.dram_tensor("w1", [h, c], F32, kind="ExternalInput")
    b1_in = nc.dram_tensor("b1", [h], F32, kind="ExternalInput")
    w2_in = nc.dram_tensor("w2", [c, h], F32, kind="ExternalInput")
    b2_in = nc.dram_tensor("b2", [c], F32, kind="ExternalInput")
    y_out = nc.dram_tensor("y", [rows, c], F32, kind="ExternalOutput")

    ct = c // 128   # c in 128-blocks
    ht = h // 128   # h in 128-blocks
    groups = [list(range(n_cores))]

    with tile.TileContext(nc) as tc, ExitStack() as top:
        consts = top.enter_context(tc.tile_pool(name="consts", bufs=1))
        scal = top.enter_context(tc.tile_pool(name="scal", bufs=1))
        dram = top.enter_context(tc.tile_pool(name="dram", bufs=1, space="DRAM"))

        ident_f = consts.tile([128, 128], F32)
        from concourse import masks
        masks.make_identity(nc, ident_f[:])

        # b1 as (128, ht): b1_sb[p, jb] = b1[jb*128 + p]
        b1_sb = consts.tile([128, ht], F32)
        nc.sync.dma_start(
            out=b1_sb[:], in_=b1_in.ap().rearrange("(a b) -> b a", b=128)
        )

        magic_b = consts.tile([128, 1], F32)
        nc.vector.memset(magic_b[:], MAGIC)

        # h scratch in DRAM, transposed + fp16: (h, rows)
        h_dram = dram.tile([h, rows], F16)
        # quantized-natural staging (bf16) for the bulk DMA transpose-loads.
        # Per-block SBUF->SBUF xbar transposes degrade into 256B packets on a
        # single HWDGE ring (measured ~25x DMA throughput collapse); bulk
        # DRAM->SBUF transpose-loads with contiguous >=1KB source rows run at
        # ~280GB/s, so all three transposed operands bounce through DRAM.
        w1q_dram = dram.tile([h, c], BF16, tag="w1qd")
        xq_dram = dram.tile([rows, c], BF16, tag="xqd")
        w2q_dram = dram.tile([c, h], BF16, tag="w2qd")
        # collective bounce buffers (DRAM, non-IO)
        arw_in = dram.tile([2, 1], F32, tag="arwi")
        arx_in = dram.tile([1, 1], F32, tag="arxi")
        arx_out = dram.tile([1, 1], F32, tag="arxo")
        arh_in = dram.tile([1, 1], F32, tag="arhi")
        arh_out = dram.tile([1, 1], F32, tag="arho")

        def _preduce(acc, psR, tag):
            """[128,1] partition-max -> [1,1] via PE transpose + DVE reduce."""
            pt = psR.tile([1, 128], F32, tag=tag)
            nc.tensor.matmul(
                pt[:], lhsT=acc[:], rhs=ident_f[:], start=True, stop=True
            )
            out = scal.tile([1, 1], F32, name=tag + "_r")
            nc.vector.tensor_reduce(
                out=out[:], in_=pt[:], axis=mybir.AxisListType.X, op=ALU.max
            )
            return out

        def _derive(bcast_src_dram, name):
            b = scal.tile([128, 1], F32, name=name + "_b")
            nc.sync.dma_start(out=b[:], in_=bcast_src_dram.to_broadcast((128, 1)))
            s = scal.tile([128, 1], F32, name="s_" + name)
            nc.vector.tensor_scalar(
                out=s[:], in0=b[:], scalar1=EPS, scalar2=float(1.0 / QP),
                op0=ALU.max, op1=ALU.mult,
            )
            inv = scal.tile([128, 1], F32, name="inv_" + name)
            nc.vector.reciprocal(out=inv[:], in_=s[:])
            return s, inv

        # ---------- phase 0: x amax scan FIRST (its AllReduce is the long
        # pole: ~50us mesh latency).  512-row tiles, DMA-bound. ----------
        xmax = scal.tile([128, 1], F32)
        nc.vector.memset(xmax[:], 0.0)
        wmax1 = scal.tile([128, 1], F32)
        nc.vector.memset(wmax1[:], 0.0)
        wmax2 = scal.tile([128, 1], F32)
        nc.vector.memset(wmax2[:], 0.0)

        with tc.tile_pool(name="scan", bufs=4) as scanp, tc.tile_pool(
            name="scant", bufs=4
        ) as scant, tc.tile_pool(name="psR", bufs=2, space="PSUM") as psR0:
            for mb in range(rows // 512):
                t = scanp.tile([128, c * 4], F32, tag="sc")
                nc.sync.dma_start(
                    out=t[:].rearrange("b (a c) -> b a c", a=4),
                    in_=x_in[mb * 512 : (mb + 1) * 512, :].rearrange(
                        "(a b) c -> b a c", b=128
                    ),
                )
                r = scant.tile([128, 1], F32, tag="scr")
                nc.vector.tensor_reduce(
                    out=r[:], in_=t[:], axis=mybir.AxisListType.X, op=ALU.max,
                    apply_absolute_value=True,
                )
                nc.vector.tensor_tensor(
                    out=xmax[:], in0=xmax[:], in1=r[:], op=ALU.max
                )
            xmax_r = _preduce(xmax, psR0, "xm")
            nc.gpsimd.dma_start(out=arx_in[:], in_=xmax_r[:])
            nc.gpsimd.collective_compute(
                "AllReduce", ALU.max, replica_groups=groups,
                ins=[arx_in.opt()], outs=[arx_out.opt()],
            )

            # w1 amax scan (same pool).  Full weights per core -> local max
            # is global, no collective.
            for rb in range(h // 512):
                t = scanp.tile([128, c * 4], F32, tag="sc")
                nc.sync.dma_start(
                    out=t[:].rearrange("b (a c) -> b a c", a=4),
                    in_=w1_in[rb * 512 : (rb + 1) * 512, :].rearrange(
                        "(a b) c -> b a c", b=128
                    ),
                )
                r = scant.tile([128, 1], F32, tag="scr")
                nc.vector.tensor_reduce(
                    out=r[:], in_=t[:], axis=mybir.AxisListType.X, op=ALU.max,
                    apply_absolute_value=True,
                )
                nc.vector.tensor_tensor(
                    out=wmax1[:], in0=wmax1[:], in1=r[:], op=ALU.max
                )
            w1max_r = _preduce(wmax1, psR0, "w1m")
            nc.gpsimd.dma_start(out=arw_in[0:1, :], in_=w1max_r[:])

        sw1, inv_sw1 = _derive(arw_in[0:1, :], "w1")

        # ---------- w1 quant -> w1q_dram -> bulk transpose-loads ----------
        # w1qT_all layout: [128(p=c%128), jg(h/512), cb(ct), 512(j within jg)]
        # so matmul lhsT for (cb, jb) = [:, jb//4, cb, (jb%4)*128:+128].
        # Pipelined per 512-row group so phase A's first jb matmuls can start
        # as soon as jg=0 lands.
        n_jg1 = h // 512
        w1_stack = ExitStack()
        w1qT_pool = w1_stack.enter_context(
            tc.tile_pool(name="w1qT", bufs=1, side="right")
        )
        w1qT_all = w1qT_pool.tile([128, n_jg1, ct, 512], BF16, tag="w1qT")

        def w1qT_ap(cb, jb):
            return w1qT_all[:, jb // 4, cb, (jb % 4) * 128 : (jb % 4 + 1) * 128]

        with tc.tile_pool(name="w1f", bufs=6) as w1f, tc.tile_pool(
            name="w1q", bufs=6
        ) as w1qp:
            for jg in range(n_jg1):
                for sub in range(4):
                    rb = jg * 4 + sub
                    wt = w1f.tile([128, c], F32, tag="w1f")
                    nc.sync.dma_start(
                        out=wt[:], in_=w1_in[rb * 128 : (rb + 1) * 128, :]
                    )
                    nc.scalar.activation(
                        out=wt[:], in_=wt[:], func=AF.Identity, bias=magic_b[:],
                        scale=inv_sw1[:],
                    )
                    wq = w1qp.tile([128, c], BF16, tag="w1q")
                    nc.vector.tensor_scalar_add(
                        out=wq[:], in0=wt[:], scalar1=-MAGIC
                    )
                    nc.sync.dma_start(
                        out=w1q_dram[rb * 128 : (rb + 1) * 128, :], in_=wq[:]
                    )
                nc.sync.dma_start_transpose(
                    out=w1qT_all[:, jg],
                    in_=w1q_dram[jg * 512 : (jg + 1) * 512, :],
                )

        sx, inv_sx = _derive(arx_out, "x")
        sxw1 = scal.tile([128, 1], F32)
        nc.vector.tensor_tensor(out=sxw1[:], in0=sx[:], in1=sw1[:], op=ALU.mult)

        hmax = scal.tile([128, 1], F32)
        nc.vector.memset(hmax[:], 0.0)

        # ---------- phase A: h.T = gelu(w1q @ xq.T * (sx*sw1) + b1) -------
        CH = min(1024, rows)
        n_ms = CH // 512
        n_chunks_a = rows // CH
        # w2 trickle plan: amax-scan strip-halves in the first half of
        # phase A, quant+store strip-halves in the second half.  Strips are
        # processed as (row-block, col-half) pieces of [128, h/2] to keep
        # the SBUF working set small.
        assert n_chunks_a >= 2
        hw2 = h // 2
        n_w2_pieces = ct * 2
        w2_scan_per_chunk = (n_w2_pieces + (n_chunks_a // 2) - 1) // (n_chunks_a // 2)
        w2_q_per_chunk = w2_scan_per_chunk
        sw2_state = {}

        with tc.tile_pool(name="w2s", bufs=3) as w2s, tc.tile_pool(
            name="w2sr", bufs=3
        ) as w2sr, tc.tile_pool(name="w2qf", bufs=2) as w2qf, tc.tile_pool(
            name="w2qb", bufs=2
        ) as w2qb, tc.tile_pool(name="xa", bufs=3) as xa, tc.tile_pool(
            name="xq", bufs=3
        ) as xqp, tc.tile_pool(name="xqt", bufs=2) as xqtp, tc.tile_pool(
            name="gs", bufs=8
        ) as gs, tc.tile_pool(name="gr", bufs=8) as gr, tc.tile_pool(
            name="psH", bufs=3 * n_ms, space="PSUM"
        ) as psH, tc.tile_pool(name="psRw", bufs=1, space="PSUM") as psRw:
            for mc in range(n_chunks_a):
                # --- w2 trickle ---
                if mc < n_chunks_a // 2:
                    for k in range(w2_scan_per_chunk):
                        pc = mc * w2_scan_per_chunk + k
                        if pc < n_w2_pieces:
                            ob, hhalf = pc // 2, pc % 2
                            wt = w2s.tile([128, hw2], F32, tag="w2s")
                            nc.sync.dma_start(
                                out=wt[:],
                                in_=w2_in[ob * 128 : (ob + 1) * 128,
                                          hhalf * hw2 : (hhalf + 1) * hw2],
                            )
                            wr = w2sr.tile([128, 1], F32, tag="w2sr")
                            nc.vector.tensor_reduce(
                                out=wr[:], in_=wt[:], axis=mybir.AxisListType.X,
                                op=ALU.max, apply_absolute_value=True,
                            )
                            nc.vector.tensor_tensor(
                                out=wmax2[:], in0=wmax2[:], in1=wr[:], op=ALU.max
                            )
                else:
                    if "inv_sw2" not in sw2_state:
                        w2max_r = _preduce(wmax2, psRw, "w2m")
                        nc.gpsimd.dma_start(out=arw_in[1:2, :], in_=w2max_r[:])
                        sw2_state["sw2"], sw2_state["inv_sw2"] = _derive(
                            arw_in[1:2, :], "w2"
                        )
                    inv_sw2 = sw2_state["inv_sw2"]
                    for k in range(w2_q_per_chunk):
                        pc = (mc - n_chunks_a // 2) * w2_q_per_chunk + k
                        if pc < n_w2_pieces:
                            ob, hhalf = pc // 2, pc % 2
                            wt = w2qf.tile([128, hw2], F32, tag="w2qf")
                            nc.sync.dma_start(
                                out=wt[:],
                                in_=w2_in[ob * 128 : (ob + 1) * 128,
                                          hhalf * hw2 : (hhalf + 1) * hw2],
                            )
                            nc.scalar.activation(
                                out=wt[:], in_=wt[:], func=AF.Identity,
                                bias=magic_b[:], scale=inv_sw2[:],
                            )
                            wq = w2qb.tile([128, hw2], BF16, tag="w2qb")
                            nc.vector.tensor_scalar_add(
                                out=wq[:], in0=wt[:], scalar1=-MAGIC
                            )
                            nc.sync.dma_start(
                                out=w2q_dram[ob * 128 : (ob + 1) * 128,
                                             hhalf * hw2 : (hhalf + 1) * hw2],
                                in_=wq[:],
                            )

                # --- x quant -> xq_dram -> one bulk transpose-load ---
                for t8 in range(CH // 128):
                    m0 = mc * CH + t8 * 128
                    xt = xa.tile([128, c], F32, tag="xa")
                    nc.sync.dma_start(out=xt[:], in_=x_in[m0 : m0 + 128, :])
                    nc.scalar.activation(
                        out=xt[:], in_=xt[:], func=AF.Identity, bias=magic_b[:],
                        scale=inv_sx[:],
                    )
                    xq = xqp.tile([128, c], BF16, tag="xq")
                    nc.vector.tensor_scalar_add(out=xq[:], in0=xt[:], scalar1=-MAGIC)
                    nc.sync.dma_start(
                        out=xq_dram[m0 : m0 + 128, :], in_=xq[:]
                    )
                xqT = xqtp.tile([128, ct, CH], BF16, tag="xqT", name=f"xqT{mc}")
                nc.sync.dma_start_transpose(
                    out=xqT[:],
                    in_=xq_dram[mc * CH : (mc + 1) * CH, :],
                )

                # --- matmul over j blocks; gelu; amax; store h.T fp16 ---
                for jb in range(ht):
                    phs = [
                        psH.tile([128, 512], F32, tag="psH", name=f"psH{mc}_{jb}_{i}")
                        for i in range(n_ms)
                    ]
                    prev = None
                    for cb in range(ct):
                        for ms in range(n_ms):
                            mmi = nc.tensor.matmul(
                                phs[ms][:],
                                lhsT=w1qT_ap(cb, jb),
                                rhs=xqT[:, cb, ms * 512 : (ms + 1) * 512],
                                start=(cb == 0),
                                stop=(cb == ct - 1),
                            )
                            if prev is not None:
                                _add_dep(mmi.ins, prev.ins, sync=False,
                                         reason="ldw-order")
                            prev = mmi
                    for ms in range(n_ms):
                        ph = phs[ms]
                        g = gs.tile([128, 512], F16, tag="gs")
                        if gelu == "Erf":
                            hh = gs.tile([128, 512], F32, tag="gh")
                            nc.scalar.activation(
                                out=hh[:], in_=ph[:], func=AF.Identity,
                                bias=b1_sb[:, jb : jb + 1], scale=sxw1[:],
                            )
                            e = gs.tile([128, 512], F32, tag="ge")
                            nc.scalar.activation(
                                out=e[:], in_=hh[:], func=AF.Erf, bias=0.0,
                                scale=float(1.0 / np.sqrt(2.0)),
                            )
                            nc.vector.tensor_scalar(
                                out=e[:], in0=e[:], scalar1=0.5, scalar2=0.5,
                                op0=ALU.mult, op1=ALU.add,
                            )
                            nc.vector.tensor_tensor(
                                out=g[:], in0=e[:], in1=hh[:], op=ALU.mult
                            )
                        else:
                            nc.scalar.activation(
                                out=g[:], in_=ph[:], func=getattr(AF, gelu),
                                bias=b1_sb[:, jb : jb + 1], scale=sxw1[:],
                            )
                        r = gr.tile([128, 1], F32, tag="gr")
                        nc.vector.tensor_reduce(
                            out=r[:], in_=g[:], axis=mybir.AxisListType.X,
                            op=ALU.max, apply_absolute_value=True,
                        )
                        nc.vector.tensor_tensor(
                            out=hmax[:], in0=hmax[:], in1=r[:], op=ALU.max
                        )
                        m0 = mc * CH + ms * 512
                        nc.sync.dma_start(
                            out=h_dram[jb * 128 : (jb + 1) * 128, m0 : m0 + 512],
                            in_=g[:],
                        )

        w1_stack.close()
        sw2 = sw2_state["sw2"]

        # ---------- h scale: AllReduce ----------
        with tc.tile_pool(name="psRh", bufs=2, space="PSUM") as psRh:
            hmax_r = _preduce(hmax, psRh, "hm")
        nc.gpsimd.dma_start(out=arh_in[:], in_=hmax_r[:])
        nc.gpsimd.collective_compute(
            "AllReduce", ALU.max, replica_groups=groups,
            ins=[arh_in.opt()], outs=[arh_out.opt()],
        )

        # ---------- w2qT materialization: bulk DMA transpose-loads --------
        # (rides under the AllReduce above)
        w2_stack = ExitStack()
        w2qT_pool = w2_stack.enter_context(tc.tile_pool(name="w2qT", bufs=1))
        w2qTs = w2qT_pool.tile([128, ht, c], BF16, tag="w2qTs")
        for jg in range(ht // 4):
            nc.sync.dma_start_transpose(
                out=w2qTs[:, jg * 4 : (jg + 1) * 4, :],
                in_=w2q_dram[:, jg * 512 : (jg + 1) * 512],
            )

        sh, inv_sh = _derive(arh_out, "h")
        shw2 = scal.tile([128, 1], F32)
        nc.vector.tensor_tensor(out=shw2[:], in0=sh[:], in1=sw2[:], op=ALU.mult)

        # ---------- phase B: y = hq.T.T @ w2q.T * (sh*sw2) + b2 ----------
        n_chunk = rows // 512
        with tc.tile_pool(name="b2p", bufs=1) as b2p, tc.tile_pool(
            name="hb", bufs=8
        ) as hb, tc.tile_pool(name="hf", bufs=4) as hf, tc.tile_pool(
            name="hqt", bufs=2
        ) as hqtp, tc.tile_pool(name="ys", bufs=4) as ys, tc.tile_pool(
            name="psY", bufs=3 * (c // 512), space="PSUM"
        ) as psY:
            b2_b = b2p.tile([128, c], F32)
            nc.sync.dma_start(
                out=b2_b[:],
                in_=b2_in.ap().rearrange("(o a) -> o a", o=1).to_broadcast((128, c)),
            )

            for mc in range(n_chunk):
                hqT = hqtp.tile([128, ht * 512], BF16, tag="hqT")
                for jb in range(ht):
                    th = hb.tile([128, 512], F16, tag="hb")
                    nc.sync.dma_start(
                        out=th[:],
                        in_=h_dram[jb * 128 : (jb + 1) * 128,
                                   mc * 512 : (mc + 1) * 512],
                    )
                    tf = hf.tile([128, 512], F32, tag="hf")
                    nc.scalar.activation(
                        out=tf[:], in_=th[:], func=AF.Identity, bias=magic_b[:],
                        scale=inv_sh[:],
                    )
                    nc.vector.tensor_scalar_add(
                        out=hqT[:, jb * 512 : (jb + 1) * 512], in0=tf[:],
                        scalar1=-MAGIC,
                    )
                n_ob = c // 512
                for ms in range(4):
                    pys = [
                        psY.tile([128, 512], F32, tag="psY", name=f"psY{mc}_{ms}_{i}")
                        for i in range(n_ob)
                    ]
                    prev = None
                    for jb in range(ht):
                        for ob in range(n_ob):
                            mmi = nc.tensor.matmul(
                                pys[ob][:],
                                lhsT=hqT[:, jb * 512 + ms * 128 :
                                         jb * 512 + (ms + 1) * 128],
                                rhs=w2qTs[:, jb, ob * 512 : (ob + 1) * 512],
                                start=(jb == 0),
                                stop=(jb == ht - 1),
                            )
                            if prev is not None:
                                _add_dep(mmi.ins, prev.ins, sync=False,
                                         reason="ldw-order")
                            prev = mmi
                    for ob in range(n_ob):
                        yt = ys.tile([128, 512], F32, tag="ys")
                        nc.vector.scalar_tensor_tensor(
                            out=yt[:], in0=pys[ob][:], scalar=shw2[:],
                            in1=b2_b[:, ob * 512 : (ob + 1) * 512],
                            op0=ALU.mult, op1=ALU.add,
                        )
                        m0 = mc * 512 + ms * 128
                        nc.sync.dma_start(
                            out=y_out[m0 : m0 + 128, ob * 512 : (ob + 1) * 512],
                            in_=yt[:],
                        )

        w2_stack.close()

    if split_waits:
        _split_matmul_waits(nc)
        _dedup_ldweights(nc)
    return nc


_CACHED = {}


def _get_nc(rows, c, h, n_cores, gelu):
    key = (rows, c, h, n_cores, gelu)
    if key not in _CACHED:
        _CACHED[key] = build_nc(rows=rows, c=c, h=h, n_cores=n_cores, gelu=gelu)
    return _CACHED[key]


def run(inputs, trace=False, gelu="Gelu", n_cores=N_CORES):
    x = np.asarray(inputs["x"], np.float32)
    w1 = np.ascontiguousarray(np.asarray(inputs["w1"], np.float32))
    b1 = np.ascontiguousarray(np.asarray(inputs["b1"], np.float32))
    w2 = np.ascontiguousarray(np.asarray(inputs["w2"], np.float32))
    b2 = np.ascontiguousarray(np.asarray(inputs["b2"], np.float32))
    b_, s_, c_ = x.shape
    h_ = w1.shape[0]
    x2d = np.ascontiguousarray(x.reshape(-1, c_))
    rows = x2d.shape[0] // n_cores
    nc = _get_nc(rows, c_, h_, n_cores, gelu)
    in_maps = [
        {
            "x": np.ascontiguousarray(x2d[i * rows : (i + 1) * rows]),
            "w1": w1,
            "b1": b1,
            "w2": w2,
            "b2": b2,
        }
        for i in range(n_cores)
    ]
    res = run_bass_kernel_spmd(nc, in_maps, list(range(n_cores)), trace=trace)
    y2d = np.concatenate([r["y"] for r in res.results], axis=0)
    return y2d.reshape(b_, s_, c_).astype(np.float32), res


def kernel(x, w1, b1, w2, b2):
    y, _ = run({"x": x, "w1": w1, "b1": b1, "w2": w2, "b2": b2})
    return y
